# revision 22
# baseline (speedup 1.0000x reference)
"""Trainium2 Bass kernel for nn_BaselineModel_35175782154746 (dense transformer
block with SiLU attention + relative-position bias).

Sharding: 8 NeuronCores = 4 batches x 2 head-groups (8 heads each).
Each core computes, for its (batch b, head-group g):
    U, Q, K, V projections (columns g*1024:(g+1)*1024 of Wu/Wq/Wk/Wv),
    SiLU attention with rel-pos bias for its 8 heads,
    gated = out * U, partial = gated @ Wf2[g*1024:(g+1)*1024, :].
Host reduces: out[b] = partial[2b] + partial[2b+1] + bf2.

All matmuls run with bf16 operands (fp32 PSUM accumulation) at N=512 moving
dim — the TensorEngine's full-rate path. Layouts keep the contraction dim on
SBUF partitions (inputs pre-transposed on host). The rel-pos bias is added in
PSUM via an identity-matmul of a host-built shifted table (pre-divided by the
attention scale so ACT's native scale finishes scores = silu(scale*(QK+bias));
for the causal variant the mask is folded into that table as -1e5, which silu
maps to an exact 0.0 in fp32). A dense-mask fallback variant handles any
non-causal attn_mask exactly.
"""

import sys
import os

for _p in ("/root/.axon_site/_ro/trn_rl_repo", "/opt/trn_rl_repo"):
    if os.path.isdir(_p) and _p not in sys.path:
        sys.path.append(_p)

import numpy as np

import concourse.bass as bass
import concourse.mybir as mybir
import concourse.tile as tile
from concourse import bacc
from concourse.bass_utils import run_bass_kernel_spmd

B, S, H, NH, MAXLEN = 4, 1024, 2048, 16, 1024
HD = H // NH            # 128
NHL = 8                 # heads per core (local)
HGRP = 2                # head groups
NCORES = 8
KT16 = H // 128         # 16 k-tiles for the H contraction
SCALE = float(HD) ** -0.5

f32 = mybir.dt.float32
f32r = mybir.dt.float32r
bf16 = mybir.dt.bfloat16
SILU = mybir.ActivationFunctionType.Silu
MULT = mybir.AluOpType.mult
ADD = mybir.AluOpType.add

TRACE = False
LAST_EXEC_NS = None
MM_DT = "bf16"          # "bf16" or "f32r" matmul operand dtype
_CACHE = {}


def _build(causal: bool, mm_dt=None):
    mmdt = {"bf16": bf16, "f32r": f32r}[mm_dt or MM_DT]
    nc = bacc.Bacc("TRN2", target_bir_lowering=False, debug=False,
                   num_devices=NCORES)

    def din(name, shape, dt=f32):
        return nc.dram_tensor(name, shape, dt, kind="ExternalInput").ap()

    qT = din("qT", [H, S], mmdt)
    kT = din("kT", [H, S], mmdt)
    vT = din("vT", [H, S], mmdt)
    wq = din("wq", [H, NHL * HD], mmdt)
    wk = din("wk", [H, NHL * HD], mmdt)
    wv = din("wv", [H, NHL * HD], mmdt)
    wu = din("wu", [H, NHL * HD], mmdt)
    wf2 = din("wf2", [NHL * HD, H], mmdt)
    bq = din("bq", [128, NHL])
    bk = din("bk", [128, NHL])
    bu = din("bu", [128, NHL])
    bv = din("bv", [1, NHL * HD], mmdt)
    ones1 = din("ones1", [1, 128], mmdt)
    atab = din("atab", [NHL, 128, 2047], bf16)
    if not causal:
        maskf = din("maskf", [128, NHL, S], bf16)
    out = nc.dram_tensor("out", [S, H], f32, kind="ExternalOutput").ap()

    with tile.TileContext(nc) as tc:
        with (
            tc.tile_pool(name="const", bufs=1) as constp,
            tc.tile_pool(name="gatedp", bufs=1) as gatedp,
        ):
            bq_t = constp.tile([128, NHL], f32, tag="bq")
            bk_t = constp.tile([128, NHL], f32, tag="bk")
            bu_t = constp.tile([128, NHL], f32, tag="bu")
            bv_t = constp.tile([1, NHL * HD], mmdt, tag="bv")
            ones_t = constp.tile([1, 128], mmdt, tag="ones1")

            gatedT = gatedp.tile([128, NHL, S], mmdt, tag="gatedT")
            wf2r = wf2.rearrange("(cb p) n -> p cb n", p=128)

            with tc.tile_pool(name="attres", bufs=1) as attres:
                UT = attres.tile([128, NHL, S], bf16, tag="UT")
                QT = attres.tile([128, NHL, S], mmdt, tag="QT")
                KTt = attres.tile([128, NHL, S], mmdt, tag="KT")
                V = attres.tile([128, NHL, S], mmdt, tag="V")
                at_tiles = [attres.tile([128, 2047], bf16,
                                        tag=f"atab{h}", name=f"atab{h}")
                            for h in range(NHL)]
                if not causal:
                    mask_t = attres.tile([128, NHL, S], bf16, tag="mask")

                with tc.tile_pool(name="inres", bufs=1) as inres:
                    qres = inres.tile([128, KT16, S], mmdt, tag="qres")
                    kres = inres.tile([128, KT16, S], mmdt, tag="kres")
                    # vres shares kres's slot: V phase runs first, K phase is
                    # last, so the kres load lands during U/Q with no stall.
                    vres = inres.tile([128, KT16, S], mmdt, tag="kres",
                                      name="vres")
                    for k in range(4):
                        nc.sync.dma_start(vres[:, k, :],
                                          vT[k * 128:(k + 1) * 128, :])
                    for k in range(4, KT16):
                        nc.sync.dma_start(vres[:, k, :],
                                          vT[k * 128:(k + 1) * 128, :])
                        nc.sync.dma_start(qres[:, k - 4, :],
                                          qT[(k - 4) * 128:(k - 3) * 128, :])
                    for k in range(KT16 - 4, KT16):
                        nc.sync.dma_start(qres[:, k, :],
                                          qT[k * 128:(k + 1) * 128, :])
                    nc.sync.dma_start(bv_t[:], bv[:])
                    nc.sync.dma_start(ones_t[:], ones1[:])
                    nc.sync.dma_start(bu_t[:], bu[:])
                    nc.sync.dma_start(bq_t[:], bq[:])
                    nc.sync.dma_start(bk_t[:], bk[:])
                    for k in range(KT16):
                        nc.sync.dma_start(kres[:, k, :],
                                          kT[k * 128:(k + 1) * 128, :])
                    for h in range(NHL):
                        nc.sync.dma_start(at_tiles[h][:], atab[h])
                    if not causal:
                        nc.sync.dma_start(mask_t[:], maskf[:])

                    with (
                        tc.tile_pool(name="win", bufs=6 if causal else 4) as winp,
                        tc.tile_pool(name="pps", bufs=1, space="PSUM") as ppsum,
                    ):
                        # ---- projection V (natural layout [S, NHL*HD]) ----
                        for ch in range(2):
                            ps = [ppsum.tile([128, 512], f32, tag=f"pp{sb}",
                                             name=f"ppv{sb}")
                                  for sb in range(8)]
                            for k in range(KT16):
                                wt = winp.tile([128, 512], mmdt, tag="wvin")
                                nc.gpsimd.dma_start(
                                    wt[:], wv[k * 128:(k + 1) * 128,
                                              ch * 512:(ch + 1) * 512])
                                for sb in range(8):
                                    nc.tensor.matmul(
                                        ps[sb][:],
                                        lhsT=vres[:, k, sb * 128:(sb + 1) * 128],
                                        rhs=wt[:],
                                        start=(k == 0), stop=False)
                            for sb in range(8):
                                nc.tensor.matmul(
                                    ps[sb][:],
                                    lhsT=ones_t[:],
                                    rhs=bv_t[:, ch * 512:(ch + 1) * 512],
                                    start=False, stop=True)
                                nc.scalar.activation(
                                    V[:, sb, ch * 512:(ch + 1) * 512],
                                    ps[sb][:], SILU)

                        # ---- projections U, Q, K ([HD, S] transposed) ----
                        for wdram, xres, btile, outtile in (
                            (wu, qres, bu_t, UT),
                            (wq, qres, bq_t, QT),
                            (wk, kres, bk_t, KTt),
                        ):
                            for ih in range(2):
                                ps = [ppsum.tile([128, 512], f32, tag=f"pp{h}",
                                                 name=f"pp{h}")
                                      for h in range(NHL)]
                                for k in range(KT16):
                                    wt = winp.tile([128, NHL * HD], mmdt,
                                                   tag="win")
                                    nc.gpsimd.dma_start(
                                        wt[:], wdram[k * 128:(k + 1) * 128, :])
                                    for h in range(NHL):
                                        nc.tensor.matmul(
                                            ps[h][:],
                                            lhsT=wt[:, h * HD:(h + 1) * HD],
                                            rhs=xres[:, k,
                                                     ih * 512:(ih + 1) * 512],
                                            start=(k == 0),
                                            stop=(k == KT16 - 1))
                                for h in range(NHL):
                                    nc.scalar.activation(
                                        outtile[:, h, ih * 512:(ih + 1) * 512],
                                        ps[h][:], SILU, bias=btile[:, h:h + 1])

                # ---- attention (ih-outer) with f2 sb0-3 interleaved into
                # the ih=1 pass; f2 sb4-7 after ----
                with (
                    tc.tile_pool(name="attnp", bufs=4) as attnp,
                    tc.tile_pool(name="psav", bufs=2, space="PSUM") as psav,
                    tc.tile_pool(name="pssc", bufs=4, space="PSUM") as pssc,
                    tc.tile_pool(name="psf2", bufs=2, space="PSUM") as psf2,
                    tc.tile_pool(name="w2p", bufs=4) as w2p,
                    tc.tile_pool(name="stgp", bufs=3) as stgp,
                ):
                    def emit_attention(h, ih):
                        njb = (4 * ih + 4) if causal else 8
                        at = at_tiles[h]
                        avp = psav.tile([128, 512], f32, tag="av",
                                        name=f"av{h}_{ih}")
                        chunks = [list(range(j, min(j + 2, njb)))
                                  for j in range(0, njb, 2)]
                        att_tiles = {}

                        def emit_scores(ch_):
                            for jb in ch_:
                                scp = pssc.tile([128, 512], f32, tag="sc",
                                                name=f"sc{h}_{ih}_{jb}")
                                nc.tensor.matmul(
                                    scp[:],
                                    lhsT=KTt[:, h, jb * 128:(jb + 1) * 128],
                                    rhs=QT[:, h, ih * 512:(ih + 1) * 512],
                                    start=True, stop=True)
                                att = attnp.tile([128, 512], mmdt, tag="attn",
                                                 name=f"at{h}_{ih}_{jb}")
                                d0 = ih * 512 - jb * 128 + MAXLEN - 1
                                nc.vector.scalar_tensor_tensor(
                                    att[:], scp[:], SCALE, at[:, d0:d0 + 512],
                                    op0=MULT, op1=ADD)
                                nc.scalar.activation(att[:], att[:], SILU)
                                if not causal:
                                    nc.vector.tensor_mul(
                                        att[:], att[:],
                                        mask_t[:, jb, ih * 512:(ih + 1) * 512])
                                att_tiles[jb] = att

                        emit_scores(chunks[0])
                        for ci, ch_ in enumerate(chunks):
                            if ci + 1 < len(chunks):
                                emit_scores(chunks[ci + 1])
                            for jb in ch_:
                                nc.tensor.matmul(
                                    avp[:],
                                    lhsT=V[:, jb, h * HD:(h + 1) * HD],
                                    rhs=att_tiles.pop(jb)[:],
                                    start=(jb == 0), stop=(jb == njb - 1))
                        nc.vector.tensor_mul(
                            gatedT[:, h, ih * 512:(ih + 1) * 512],
                            avp[:],
                            UT[:, h, ih * 512:(ih + 1) * 512])

                    def emit_f2_block(w2t, n, sb):
                        ps = psf2.tile([128, 512], f32, tag="f2",
                                       name=f"f2_{n}_{sb}")
                        for cb in range(NHL):
                            nc.tensor.matmul(
                                ps[:],
                                lhsT=gatedT[:, cb, sb * 128:(sb + 1) * 128],
                                rhs=w2t[:, cb, :],
                                start=(cb == 0), stop=(cb == NHL - 1))
                        st = stgp.tile([128, 512], f32, tag="st",
                                       name=f"st{n}_{sb}")
                        nc.vector.tensor_copy(st[:], ps[:])
                        nc.sync.dma_start(
                            out[sb * 128:(sb + 1) * 128,
                                n * 512:(n + 1) * 512], st[:])

                    for h in range(NHL):
                        emit_attention(h, 0)

                    w2a = []
                    for n in range(4):
                        t = w2p.tile([128, NHL, 512], mmdt, tag="w2",
                                     name=f"w2a{n}")
                        nc.sync.dma_start(t[:],
                                          wf2r[:, :, n * 512:(n + 1) * 512])
                        w2a.append(t)

                    fa = [(n, sb) for n in range(4) for sb in range(4)]
                    w2b = []
                    for i in range(NHL):
                        emit_attention(i, 1)
                        for n, sb in fa[2 * i:2 * (i + 1)]:
                            emit_f2_block(w2a[n], n, sb)
                        if i % 2 == 1:
                            # column i//2 of part A is done - prefetch its
                            # part-B replacement into the freed slot
                            t = w2p.tile([128, NHL, 512], mmdt, tag="w2",
                                         name=f"w2b{i // 2}")
                            nc.gpsimd.dma_start(
                                t[:], wf2r[:, :, (i // 2) * 512:
                                           (i // 2 + 1) * 512])
                            w2b.append(t)

                    for n in range(4):
                        for sb in range(4, 8):
                            emit_f2_block(w2b[n], n, sb)

    nc.compile()
    return nc


def _host_shards(query, key, value, attn_mask, Wq, bq, Wk, bk, Wv, bv,
                 Wu, bu, Wf2, rel_table, causal, mm_dt=None):
    """Build the per-core input maps."""
    import ml_dtypes
    npdt = (np.dtype(ml_dtypes.bfloat16) if (mm_dt or MM_DT) == "bf16"
            else np.float32)
    _ONES128 = np.ones((1, 128)).astype(npdt)
    in_maps = []
    # precompute per-head-group weight slices once (shared by 4 cores each)
    gdata = []
    for g in range(HGRP):
        c0, c1 = g * NHL * HD, (g + 1) * NHL * HD
        wq_c = np.ascontiguousarray(Wq[:, c0:c1]).astype(npdt)
        wk_c = np.ascontiguousarray(Wk[:, c0:c1]).astype(npdt)
        wv_c = np.ascontiguousarray(Wv[:, c0:c1]).astype(npdt)
        wu_c = np.ascontiguousarray(Wu[:, c0:c1]).astype(npdt)
        wf2_c = np.ascontiguousarray(Wf2[c0:c1, :]).astype(npdt)
        bq_c = np.ascontiguousarray(bq[c0:c1].reshape(NHL, 128).T)
        bk_c = np.ascontiguousarray(bk[c0:c1].reshape(NHL, 128).T)
        bu_c = np.ascontiguousarray(bu[c0:c1].reshape(NHL, 128).T)
        bv_c = np.ascontiguousarray(bv[c0:c1][None, :]).astype(npdt)
        # atab[h, r, y] = table[y - r, g*NHL + h]; for the causal variant the
        # table is pre-divided by SCALE and masked entries (m < MAXLEN-1,
        # i.e. key index > query index) are -1e5 so silu gives exactly 0.
        y = np.arange(2047)[None, :]
        r = np.arange(128)[:, None]
        idx = y - r                      # [128, 2047]
        valid = (idx >= 0) & (idx <= 2 * MAXLEN - 2)
        idxc = np.clip(idx, 0, 2 * MAXLEN - 2)
        cols = rel_table[:, g * NHL:(g + 1) * NHL]   # [2047, NHL]
        import ml_dtypes as _mld
        if causal:
            cols = np.where(np.arange(2047)[:, None] >= MAXLEN - 1, cols,
                            np.float32(-1e5))
            at = np.where(valid[:, :, None], cols[idxc], np.float32(-1e5))
        else:
            at = cols[idxc] * valid[:, :, None]
        atab_c = np.ascontiguousarray(
            at.transpose(2, 0, 1)).astype(_mld.bfloat16)
        gdata.append((wq_c, wk_c, wv_c, wu_c, wf2_c, bq_c, bk_c, bu_c,
                      bv_c, atab_c))

    for c in range(NCORES):
        b, g = c // HGRP, c % HGRP
        (wq_c, wk_c, wv_c, wu_c, wf2_c, bq_c, bk_c, bu_c, bv_c,
         atab_c) = gdata[g]
        m = {
            "qT": np.ascontiguousarray(query[b].T).astype(npdt),
            "kT": np.ascontiguousarray(key[b].T).astype(npdt),
            "vT": np.ascontiguousarray(value[b].T).astype(npdt),
            "wq": wq_c, "wk": wk_c, "wv": wv_c, "wu": wu_c, "wf2": wf2_c,
            "bq": bq_c, "bk": bk_c, "bu": bu_c, "bv": bv_c, "atab": atab_c,
            "ones1": _ONES128,
        }
        mb = attn_mask[b]
        if not causal:
            import ml_dtypes as _mld
            mf = np.empty((128, NHL, S), _mld.bfloat16)
            for jb in range(8):
                mf[:, jb, :] = mb[:, jb * 128:(jb + 1) * 128].T
            m["maskf"] = mf
        in_maps.append(m)
    return in_maps


def kernel(query, key, value, attn_mask, Wq, bq, Wk, bk, Wv, bv, Wu, bu,
           Wf2, bf2, rel_table):
    global LAST_EXEC_NS
    query = np.asarray(query, np.float32)
    key = np.asarray(key, np.float32)
    value = np.asarray(value, np.float32)
    attn_mask = np.asarray(attn_mask, bool)
    Wq, bq = np.asarray(Wq, np.float32), np.asarray(bq, np.float32)
    Wk, bk = np.asarray(Wk, np.float32), np.asarray(bk, np.float32)
    Wv, bv = np.asarray(Wv, np.float32), np.asarray(bv, np.float32)
    Wu, bu = np.asarray(Wu, np.float32), np.asarray(bu, np.float32)
    Wf2, bf2 = np.asarray(Wf2, np.float32), np.asarray(bf2, np.float32)
    rel_table = np.asarray(rel_table, np.float32)

    tril = np.tril(np.ones((S, S), bool))
    causal = all(np.array_equal(attn_mask[b], tril) for b in range(B))

    key_ = (causal, MM_DT)
    if key_ not in _CACHE:
        _CACHE[key_] = _build(causal)
    nc = _CACHE[key_]

    in_maps = _host_shards(query, key, value, attn_mask, Wq, bq, Wk, bk,
                           Wv, bv, Wu, bu, Wf2, rel_table, causal)
    res = run_bass_kernel_spmd(nc, in_maps, list(range(NCORES)), trace=TRACE)
    if res.exec_time_ns is not None:
        LAST_EXEC_NS = res.exec_time_ns

    outp = np.empty((B, S, H), np.float32)
    for b in range(B):
        outp[b] = (res.results[2 * b]["out"] + res.results[2 * b + 1]["out"]
                   + bf2[None, :])
    return outp


# revision 23
# speedup vs baseline: 1.0123x; 1.0123x over previous
"""Trainium2 Bass kernel for nn_BaselineModel_35175782154746 (dense transformer
block with SiLU attention + relative-position bias).

Sharding: 8 NeuronCores = 4 batches x 2 head-groups (8 heads each).
Each core computes, for its (batch b, head-group g):
    U, Q, K, V projections (columns g*1024:(g+1)*1024 of Wu/Wq/Wk/Wv),
    SiLU attention with rel-pos bias for its 8 heads,
    gated = out * U, partial = gated @ Wf2[g*1024:(g+1)*1024, :].
Host reduces: out[b] = partial[2b] + partial[2b+1] + bf2.

All matmuls run with bf16 operands (fp32 PSUM accumulation) at N=512 moving
dim — the TensorEngine's full-rate path. Layouts keep the contraction dim on
SBUF partitions (inputs pre-transposed on host). The rel-pos bias is added in
PSUM via an identity-matmul of a host-built shifted table (pre-divided by the
attention scale so ACT's native scale finishes scores = silu(scale*(QK+bias));
for the causal variant the mask is folded into that table as -1e5, which silu
maps to an exact 0.0 in fp32). A dense-mask fallback variant handles any
non-causal attn_mask exactly.
"""

import sys
import os

for _p in ("/root/.axon_site/_ro/trn_rl_repo", "/opt/trn_rl_repo"):
    if os.path.isdir(_p) and _p not in sys.path:
        sys.path.append(_p)

import numpy as np

import concourse.bass as bass
import concourse.mybir as mybir
import concourse.tile as tile
from concourse import bacc
from concourse.bass_utils import run_bass_kernel_spmd

B, S, H, NH, MAXLEN = 4, 1024, 2048, 16, 1024
HD = H // NH            # 128
NHL = 8                 # heads per core (local)
HGRP = 2                # head groups
NCORES = 8
KT16 = H // 128         # 16 k-tiles for the H contraction
SCALE = float(HD) ** -0.5

f32 = mybir.dt.float32
f32r = mybir.dt.float32r
bf16 = mybir.dt.bfloat16
SILU = mybir.ActivationFunctionType.Silu
MULT = mybir.AluOpType.mult
ADD = mybir.AluOpType.add

TRACE = False
LAST_EXEC_NS = None
MM_DT = "bf16"          # "bf16" or "f32r" matmul operand dtype
_CACHE = {}


def _build(causal: bool, mm_dt=None):
    mmdt = {"bf16": bf16, "f32r": f32r}[mm_dt or MM_DT]
    nc = bacc.Bacc("TRN2", target_bir_lowering=False, debug=False,
                   num_devices=NCORES)

    def din(name, shape, dt=f32):
        return nc.dram_tensor(name, shape, dt, kind="ExternalInput").ap()

    qT = din("qT", [H, S], mmdt)
    kT = din("kT", [H, S], mmdt)
    vT = din("vT", [H, S], mmdt)
    wq = din("wq", [H, NHL * HD], mmdt)
    wk = din("wk", [H, NHL * HD], mmdt)
    wv = din("wv", [H, NHL * HD], mmdt)
    wu = din("wu", [H, NHL * HD], mmdt)
    wf2 = din("wf2", [NHL * HD, H], mmdt)
    bq = din("bq", [128, NHL])
    bk = din("bk", [128, NHL])
    bu = din("bu", [128, NHL])
    bv = din("bv", [1, NHL * HD], mmdt)
    ones1 = din("ones1", [1, 128], mmdt)
    atab = din("atab", [NHL, 128, 2047], bf16)
    if not causal:
        maskf = din("maskf", [128, NHL, S], bf16)
    out = nc.dram_tensor("out", [S, H], f32, kind="ExternalOutput").ap()

    with tile.TileContext(nc) as tc:
        with (
            tc.tile_pool(name="const", bufs=1) as constp,
            tc.tile_pool(name="gatedp", bufs=1) as gatedp,
        ):
            bq_t = constp.tile([128, NHL], f32, tag="bq")
            bk_t = constp.tile([128, NHL], f32, tag="bk")
            bu_t = constp.tile([128, NHL], f32, tag="bu")
            bv_t = constp.tile([1, NHL * HD], mmdt, tag="bv")
            ones_t = constp.tile([1, 128], mmdt, tag="ones1")

            gatedT = gatedp.tile([128, NHL, S], mmdt, tag="gatedT")
            wf2r = wf2.rearrange("(cb p) n -> p cb n", p=128)

            with tc.tile_pool(name="attres", bufs=1) as attres:
                UT = attres.tile([128, NHL, S], bf16, tag="UT")
                QT = attres.tile([128, NHL, S], mmdt, tag="QT")
                KTt = attres.tile([128, NHL, S], mmdt, tag="KT")
                V = attres.tile([128, NHL, S], mmdt, tag="V")
                at_tiles = [attres.tile([128, 2047], bf16,
                                        tag=f"atab{h}", name=f"atab{h}")
                            for h in range(NHL)]
                if not causal:
                    mask_t = attres.tile([128, NHL, S], bf16, tag="mask")

                with tc.tile_pool(name="inres", bufs=1) as inres:
                    qres = inres.tile([128, KT16, S], mmdt, tag="qres")
                    kres = inres.tile([128, KT16, S], mmdt, tag="kres")
                    # vres shares kres's slot: V phase runs first, K phase is
                    # last, so the kres load lands during U/Q with no stall.
                    vres = inres.tile([128, KT16, S], mmdt, tag="kres",
                                      name="vres")
                    for k in range(8):
                        nc.sync.dma_start(vres[:, k, :],
                                          vT[k * 128:(k + 1) * 128, :])
                    for k in range(8, KT16):
                        nc.sync.dma_start(vres[:, k, :],
                                          vT[k * 128:(k + 1) * 128, :])
                        nc.sync.dma_start(qres[:, k - 8, :],
                                          qT[(k - 8) * 128:(k - 7) * 128, :])
                    for k in range(8, KT16):
                        nc.sync.dma_start(qres[:, k, :],
                                          qT[k * 128:(k + 1) * 128, :])
                    nc.sync.dma_start(bv_t[:], bv[:])
                    nc.sync.dma_start(ones_t[:], ones1[:])
                    nc.sync.dma_start(bu_t[:], bu[:])
                    nc.sync.dma_start(bq_t[:], bq[:])
                    nc.sync.dma_start(bk_t[:], bk[:])
                    for k in range(KT16):
                        nc.sync.dma_start(kres[:, k, :],
                                          kT[k * 128:(k + 1) * 128, :])
                    for h in range(NHL):
                        nc.sync.dma_start(at_tiles[h][:], atab[h])
                    if not causal:
                        nc.sync.dma_start(mask_t[:], maskf[:])

                    with (
                        tc.tile_pool(name="win", bufs=6 if causal else 4) as winp,
                        tc.tile_pool(name="pps", bufs=1, space="PSUM") as ppsum,
                    ):
                        # ---- projection V (natural layout [S, NHL*HD]) ----
                        for ch in range(2):
                            ps = [ppsum.tile([128, 512], f32, tag=f"pp{sb}",
                                             name=f"ppv{sb}")
                                  for sb in range(8)]
                            for k in range(KT16):
                                wt = winp.tile([128, 512], mmdt, tag="wvin")
                                nc.gpsimd.dma_start(
                                    wt[:], wv[k * 128:(k + 1) * 128,
                                              ch * 512:(ch + 1) * 512])
                                for sb in range(8):
                                    nc.tensor.matmul(
                                        ps[sb][:],
                                        lhsT=vres[:, k, sb * 128:(sb + 1) * 128],
                                        rhs=wt[:],
                                        start=(k == 0), stop=False)
                            for sb in range(8):
                                nc.tensor.matmul(
                                    ps[sb][:],
                                    lhsT=ones_t[:],
                                    rhs=bv_t[:, ch * 512:(ch + 1) * 512],
                                    start=False, stop=True)
                                nc.scalar.activation(
                                    V[:, sb, ch * 512:(ch + 1) * 512],
                                    ps[sb][:], SILU)

                        # ---- projections U, Q, K ([HD, S] transposed) ----
                        for wdram, xres, btile, outtile in (
                            (wu, qres, bu_t, UT),
                            (wq, qres, bq_t, QT),
                            (wk, kres, bk_t, KTt),
                        ):
                            for ih in range(2):
                                ps = [ppsum.tile([128, 512], f32, tag=f"pp{h}",
                                                 name=f"pp{h}")
                                      for h in range(NHL)]
                                for k in range(KT16):
                                    wt = winp.tile([128, NHL * HD], mmdt,
                                                   tag="win")
                                    nc.gpsimd.dma_start(
                                        wt[:], wdram[k * 128:(k + 1) * 128, :])
                                    for h in range(NHL):
                                        nc.tensor.matmul(
                                            ps[h][:],
                                            lhsT=wt[:, h * HD:(h + 1) * HD],
                                            rhs=xres[:, k,
                                                     ih * 512:(ih + 1) * 512],
                                            start=(k == 0),
                                            stop=(k == KT16 - 1))
                                for h in range(NHL):
                                    nc.scalar.activation(
                                        outtile[:, h, ih * 512:(ih + 1) * 512],
                                        ps[h][:], SILU, bias=btile[:, h:h + 1])

                # ---- attention (ih-outer) with f2 sb0-3 interleaved into
                # the ih=1 pass; f2 sb4-7 after ----
                with (
                    tc.tile_pool(name="attnp", bufs=4) as attnp,
                    tc.tile_pool(name="psav", bufs=2, space="PSUM") as psav,
                    tc.tile_pool(name="pssc", bufs=4, space="PSUM") as pssc,
                    tc.tile_pool(name="psf2", bufs=2, space="PSUM") as psf2,
                    tc.tile_pool(name="w2p", bufs=4) as w2p,
                    tc.tile_pool(name="stgp", bufs=3) as stgp,
                ):
                    def emit_attention(h, ih):
                        njb = (4 * ih + 4) if causal else 8
                        at = at_tiles[h]
                        avp = psav.tile([128, 512], f32, tag="av",
                                        name=f"av{h}_{ih}")
                        chunks = [list(range(j, min(j + 2, njb)))
                                  for j in range(0, njb, 2)]
                        att_tiles = {}

                        def emit_scores(ch_):
                            for jb in ch_:
                                scp = pssc.tile([128, 512], f32, tag="sc",
                                                name=f"sc{h}_{ih}_{jb}")
                                nc.tensor.matmul(
                                    scp[:],
                                    lhsT=KTt[:, h, jb * 128:(jb + 1) * 128],
                                    rhs=QT[:, h, ih * 512:(ih + 1) * 512],
                                    start=True, stop=True)
                                att = attnp.tile([128, 512], mmdt, tag="attn",
                                                 name=f"at{h}_{ih}_{jb}")
                                d0 = ih * 512 - jb * 128 + MAXLEN - 1
                                nc.vector.scalar_tensor_tensor(
                                    att[:], scp[:], SCALE, at[:, d0:d0 + 512],
                                    op0=MULT, op1=ADD)
                                nc.scalar.activation(att[:], att[:], SILU)
                                if not causal:
                                    nc.vector.tensor_mul(
                                        att[:], att[:],
                                        mask_t[:, jb, ih * 512:(ih + 1) * 512])
                                att_tiles[jb] = att

                        emit_scores(chunks[0])
                        for ci, ch_ in enumerate(chunks):
                            if ci + 1 < len(chunks):
                                emit_scores(chunks[ci + 1])
                            for jb in ch_:
                                nc.tensor.matmul(
                                    avp[:],
                                    lhsT=V[:, jb, h * HD:(h + 1) * HD],
                                    rhs=att_tiles.pop(jb)[:],
                                    start=(jb == 0), stop=(jb == njb - 1))
                        nc.vector.tensor_mul(
                            gatedT[:, h, ih * 512:(ih + 1) * 512],
                            avp[:],
                            UT[:, h, ih * 512:(ih + 1) * 512])

                    def emit_f2_block(w2t, n, sb):
                        ps = psf2.tile([128, 512], f32, tag="f2",
                                       name=f"f2_{n}_{sb}")
                        for cb in range(NHL):
                            nc.tensor.matmul(
                                ps[:],
                                lhsT=gatedT[:, cb, sb * 128:(sb + 1) * 128],
                                rhs=w2t[:, cb, :],
                                start=(cb == 0), stop=(cb == NHL - 1))
                        st = stgp.tile([128, 512], f32, tag="st",
                                       name=f"st{n}_{sb}")
                        nc.vector.tensor_copy(st[:], ps[:])
                        nc.sync.dma_start(
                            out[sb * 128:(sb + 1) * 128,
                                n * 512:(n + 1) * 512], st[:])

                    for h in range(NHL):
                        emit_attention(h, 0)

                    w2a = []
                    for n in range(4):
                        t = w2p.tile([128, NHL, 512], mmdt, tag="w2",
                                     name=f"w2a{n}")
                        nc.sync.dma_start(t[:],
                                          wf2r[:, :, n * 512:(n + 1) * 512])
                        w2a.append(t)

                    fa = [(n, sb) for n in range(4) for sb in range(4)]
                    w2b = []
                    for i in range(NHL):
                        emit_attention(i, 1)
                        for n, sb in fa[2 * i:2 * (i + 1)]:
                            emit_f2_block(w2a[n], n, sb)
                        if i % 2 == 1:
                            # column i//2 of part A is done - prefetch its
                            # part-B replacement into the freed slot
                            t = w2p.tile([128, NHL, 512], mmdt, tag="w2",
                                         name=f"w2b{i // 2}")
                            nc.gpsimd.dma_start(
                                t[:], wf2r[:, :, (i // 2) * 512:
                                           (i // 2 + 1) * 512])
                            w2b.append(t)

                    for n in range(4):
                        for sb in range(4, 8):
                            emit_f2_block(w2b[n], n, sb)

    nc.compile()
    return nc


def _host_shards(query, key, value, attn_mask, Wq, bq, Wk, bk, Wv, bv,
                 Wu, bu, Wf2, rel_table, causal, mm_dt=None):
    """Build the per-core input maps."""
    import ml_dtypes
    npdt = (np.dtype(ml_dtypes.bfloat16) if (mm_dt or MM_DT) == "bf16"
            else np.float32)
    _ONES128 = np.ones((1, 128)).astype(npdt)
    in_maps = []
    # precompute per-head-group weight slices once (shared by 4 cores each)
    gdata = []
    for g in range(HGRP):
        c0, c1 = g * NHL * HD, (g + 1) * NHL * HD
        wq_c = np.ascontiguousarray(Wq[:, c0:c1]).astype(npdt)
        wk_c = np.ascontiguousarray(Wk[:, c0:c1]).astype(npdt)
        wv_c = np.ascontiguousarray(Wv[:, c0:c1]).astype(npdt)
        wu_c = np.ascontiguousarray(Wu[:, c0:c1]).astype(npdt)
        wf2_c = np.ascontiguousarray(Wf2[c0:c1, :]).astype(npdt)
        bq_c = np.ascontiguousarray(bq[c0:c1].reshape(NHL, 128).T)
        bk_c = np.ascontiguousarray(bk[c0:c1].reshape(NHL, 128).T)
        bu_c = np.ascontiguousarray(bu[c0:c1].reshape(NHL, 128).T)
        bv_c = np.ascontiguousarray(bv[c0:c1][None, :]).astype(npdt)
        # atab[h, r, y] = table[y - r, g*NHL + h]; for the causal variant the
        # table is pre-divided by SCALE and masked entries (m < MAXLEN-1,
        # i.e. key index > query index) are -1e5 so silu gives exactly 0.
        y = np.arange(2047)[None, :]
        r = np.arange(128)[:, None]
        idx = y - r                      # [128, 2047]
        valid = (idx >= 0) & (idx <= 2 * MAXLEN - 2)
        idxc = np.clip(idx, 0, 2 * MAXLEN - 2)
        cols = rel_table[:, g * NHL:(g + 1) * NHL]   # [2047, NHL]
        import ml_dtypes as _mld
        if causal:
            cols = np.where(np.arange(2047)[:, None] >= MAXLEN - 1, cols,
                            np.float32(-1e5))
            at = np.where(valid[:, :, None], cols[idxc], np.float32(-1e5))
        else:
            at = cols[idxc] * valid[:, :, None]
        atab_c = np.ascontiguousarray(
            at.transpose(2, 0, 1)).astype(_mld.bfloat16)
        gdata.append((wq_c, wk_c, wv_c, wu_c, wf2_c, bq_c, bk_c, bu_c,
                      bv_c, atab_c))

    for c in range(NCORES):
        b, g = c // HGRP, c % HGRP
        (wq_c, wk_c, wv_c, wu_c, wf2_c, bq_c, bk_c, bu_c, bv_c,
         atab_c) = gdata[g]
        m = {
            "qT": np.ascontiguousarray(query[b].T).astype(npdt),
            "kT": np.ascontiguousarray(key[b].T).astype(npdt),
            "vT": np.ascontiguousarray(value[b].T).astype(npdt),
            "wq": wq_c, "wk": wk_c, "wv": wv_c, "wu": wu_c, "wf2": wf2_c,
            "bq": bq_c, "bk": bk_c, "bu": bu_c, "bv": bv_c, "atab": atab_c,
            "ones1": _ONES128,
        }
        mb = attn_mask[b]
        if not causal:
            import ml_dtypes as _mld
            mf = np.empty((128, NHL, S), _mld.bfloat16)
            for jb in range(8):
                mf[:, jb, :] = mb[:, jb * 128:(jb + 1) * 128].T
            m["maskf"] = mf
        in_maps.append(m)
    return in_maps


def kernel(query, key, value, attn_mask, Wq, bq, Wk, bk, Wv, bv, Wu, bu,
           Wf2, bf2, rel_table):
    global LAST_EXEC_NS
    query = np.asarray(query, np.float32)
    key = np.asarray(key, np.float32)
    value = np.asarray(value, np.float32)
    attn_mask = np.asarray(attn_mask, bool)
    Wq, bq = np.asarray(Wq, np.float32), np.asarray(bq, np.float32)
    Wk, bk = np.asarray(Wk, np.float32), np.asarray(bk, np.float32)
    Wv, bv = np.asarray(Wv, np.float32), np.asarray(bv, np.float32)
    Wu, bu = np.asarray(Wu, np.float32), np.asarray(bu, np.float32)
    Wf2, bf2 = np.asarray(Wf2, np.float32), np.asarray(bf2, np.float32)
    rel_table = np.asarray(rel_table, np.float32)

    tril = np.tril(np.ones((S, S), bool))
    causal = all(np.array_equal(attn_mask[b], tril) for b in range(B))

    key_ = (causal, MM_DT)
    if key_ not in _CACHE:
        _CACHE[key_] = _build(causal)
    nc = _CACHE[key_]

    in_maps = _host_shards(query, key, value, attn_mask, Wq, bq, Wk, bk,
                           Wv, bv, Wu, bu, Wf2, rel_table, causal)
    res = run_bass_kernel_spmd(nc, in_maps, list(range(NCORES)), trace=TRACE)
    if res.exec_time_ns is not None:
        LAST_EXEC_NS = res.exec_time_ns

    outp = np.empty((B, S, H), np.float32)
    for b in range(B):
        outp[b] = (res.results[2 * b]["out"] + res.results[2 * b + 1]["out"]
                   + bf2[None, :])
    return outp


# revision 24
# speedup vs baseline: 1.0358x; 1.0232x over previous
"""Trainium2 Bass kernel for nn_BaselineModel_35175782154746 (dense transformer
block with SiLU attention + relative-position bias).

Sharding: 8 NeuronCores = 4 batches x 2 head-groups (8 heads each).
Each core computes, for its (batch b, head-group g):
    U, Q, K, V projections (columns g*1024:(g+1)*1024 of Wu/Wq/Wk/Wv),
    SiLU attention with rel-pos bias for its 8 heads,
    gated = out * U, partial = gated @ Wf2[g*1024:(g+1)*1024, :].
Host reduces: out[b] = partial[2b] + partial[2b+1] + bf2.

All matmuls run with bf16 operands (fp32 PSUM accumulation) at N=512 moving
dim — the TensorEngine's full-rate path. Layouts keep the contraction dim on
SBUF partitions (inputs pre-transposed on host). The rel-pos bias is added in
PSUM via an identity-matmul of a host-built shifted table (pre-divided by the
attention scale so ACT's native scale finishes scores = silu(scale*(QK+bias));
for the causal variant the mask is folded into that table as -1e5, which silu
maps to an exact 0.0 in fp32). A dense-mask fallback variant handles any
non-causal attn_mask exactly.
"""

import sys
import os

for _p in ("/root/.axon_site/_ro/trn_rl_repo", "/opt/trn_rl_repo"):
    if os.path.isdir(_p) and _p not in sys.path:
        sys.path.append(_p)

import numpy as np

import concourse.bass as bass
import concourse.mybir as mybir
import concourse.tile as tile
from concourse import bacc
from concourse.bass_utils import run_bass_kernel_spmd

B, S, H, NH, MAXLEN = 4, 1024, 2048, 16, 1024
HD = H // NH            # 128
NHL = 8                 # heads per core (local)
HGRP = 2                # head groups
NCORES = 8
KT16 = H // 128         # 16 k-tiles for the H contraction
SCALE = float(HD) ** -0.5

f32 = mybir.dt.float32
f32r = mybir.dt.float32r
bf16 = mybir.dt.bfloat16
SILU = mybir.ActivationFunctionType.Silu
MULT = mybir.AluOpType.mult
ADD = mybir.AluOpType.add

TRACE = False
LAST_EXEC_NS = None
MM_DT = "bf16"          # "bf16" or "f32r" matmul operand dtype
_CACHE = {}


def _build(causal: bool, mm_dt=None):
    mmdt = {"bf16": bf16, "f32r": f32r}[mm_dt or MM_DT]
    nc = bacc.Bacc("TRN2", target_bir_lowering=False, debug=False,
                   num_devices=NCORES)

    def din(name, shape, dt=f32):
        return nc.dram_tensor(name, shape, dt, kind="ExternalInput").ap()

    qT = din("qT", [H, S], mmdt)
    kT = din("kT", [H, S], mmdt)
    vT = din("vT", [H, S], mmdt)
    wq = din("wq", [H, NHL * HD], mmdt)
    wk = din("wk", [H, NHL * HD], mmdt)
    wv = din("wv", [H, NHL * HD], mmdt)
    wu = din("wu", [H, NHL * HD], mmdt)
    wf2 = din("wf2", [NHL * HD, H], mmdt)
    bq = din("bq", [128, NHL])
    bk = din("bk", [128, NHL])
    bu = din("bu", [128, NHL])
    bv = din("bv", [1, NHL * HD], mmdt)
    ones1 = din("ones1", [1, 128], mmdt)
    atab = din("atab", [NHL, 128, 2047], bf16)
    if not causal:
        maskf = din("maskf", [128, NHL, S], bf16)
    out = nc.dram_tensor("out", [S, H], f32, kind="ExternalOutput").ap()

    with tile.TileContext(nc) as tc:
        with (
            tc.tile_pool(name="const", bufs=1) as constp,
            tc.tile_pool(name="gatedp", bufs=1) as gatedp,
        ):
            bq_t = constp.tile([128, NHL], f32, tag="bq")
            bk_t = constp.tile([128, NHL], f32, tag="bk")
            bu_t = constp.tile([128, NHL], f32, tag="bu")
            bv_t = constp.tile([1, NHL * HD], mmdt, tag="bv")
            ones_t = constp.tile([1, 128], mmdt, tag="ones1")

            gatedT = gatedp.tile([128, NHL, S], mmdt, tag="gatedT")
            wf2r = wf2.rearrange("(cb p) n -> p cb n", p=128)

            with tc.tile_pool(name="attres", bufs=1) as attres:
                UT = attres.tile([128, NHL, S], bf16, tag="UT")
                QT = attres.tile([128, NHL, S], mmdt, tag="QT")
                KTt = attres.tile([128, NHL, S], mmdt, tag="KT")
                V = attres.tile([128, NHL, S], mmdt, tag="V")
                at_tiles = [attres.tile([128, 2047], bf16,
                                        tag=f"atab{h}", name=f"atab{h}")
                            for h in range(NHL)]
                if not causal:
                    mask_t = attres.tile([128, NHL, S], bf16, tag="mask")

                with tc.tile_pool(name="inres", bufs=1) as inres:
                    qres = inres.tile([128, KT16, S], mmdt, tag="qres")
                    kres = inres.tile([128, KT16, S], mmdt, tag="kres")
                    # vres shares qres's slot: qres's last read is the Q
                    # phase, V runs last, so the vres load lands during K.
                    vres = inres.tile([128, KT16, S], mmdt, tag="qres",
                                      name="vres")
                    for k in range(KT16):
                        nc.sync.dma_start(qres[:, k, :],
                                          qT[k * 128:(k + 1) * 128, :])
                    nc.sync.dma_start(bu_t[:], bu[:])
                    nc.sync.dma_start(bq_t[:], bq[:])
                    nc.sync.dma_start(bk_t[:], bk[:])
                    nc.sync.dma_start(bv_t[:], bv[:])
                    nc.sync.dma_start(ones_t[:], ones1[:])
                    for k in range(KT16):
                        nc.sync.dma_start(kres[:, k, :],
                                          kT[k * 128:(k + 1) * 128, :])
                    for k in range(KT16):
                        nc.sync.dma_start(vres[:, k, :],
                                          vT[k * 128:(k + 1) * 128, :])
                    for h in range(NHL):
                        nc.sync.dma_start(at_tiles[h][:], atab[h])
                    if not causal:
                        nc.sync.dma_start(mask_t[:], maskf[:])

                    with (
                        tc.tile_pool(name="win", bufs=6 if causal else 4) as winp,
                        tc.tile_pool(name="pps", bufs=1, space="PSUM") as ppsum,
                    ):
                        # ---- projections U, Q, K ([HD, S] transposed) ----
                        for wdram, xres, btile, outtile in (
                            (wu, qres, bu_t, UT),
                            (wq, qres, bq_t, QT),
                            (wk, kres, bk_t, KTt),
                        ):
                            for ih in range(2):
                                ps = [ppsum.tile([128, 512], f32, tag=f"pp{h}",
                                                 name=f"pp{h}")
                                      for h in range(NHL)]
                                for k in range(KT16):
                                    wt = winp.tile([128, NHL * HD], mmdt,
                                                   tag="win")
                                    nc.gpsimd.dma_start(
                                        wt[:], wdram[k * 128:(k + 1) * 128, :])
                                    for h in range(NHL):
                                        nc.tensor.matmul(
                                            ps[h][:],
                                            lhsT=wt[:, h * HD:(h + 1) * HD],
                                            rhs=xres[:, k,
                                                     ih * 512:(ih + 1) * 512],
                                            start=(k == 0),
                                            stop=(k == KT16 - 1))
                                for h in range(NHL):
                                    nc.scalar.activation(
                                        outtile[:, h, ih * 512:(ih + 1) * 512],
                                        ps[h][:], SILU, bias=btile[:, h:h + 1])

                        # ---- projection V (natural layout [S, NHL*HD]) ----
                        for ch in range(2):
                            ps = [ppsum.tile([128, 512], f32, tag=f"pp{sb}",
                                             name=f"ppv{sb}")
                                  for sb in range(8)]
                            for k in range(KT16):
                                wt = winp.tile([128, 512], mmdt, tag="wvin")
                                nc.gpsimd.dma_start(
                                    wt[:], wv[k * 128:(k + 1) * 128,
                                              ch * 512:(ch + 1) * 512])
                                for sb in range(8):
                                    nc.tensor.matmul(
                                        ps[sb][:],
                                        lhsT=vres[:, k, sb * 128:(sb + 1) * 128],
                                        rhs=wt[:],
                                        start=(k == 0), stop=False)
                            for sb in range(8):
                                nc.tensor.matmul(
                                    ps[sb][:],
                                    lhsT=ones_t[:],
                                    rhs=bv_t[:, ch * 512:(ch + 1) * 512],
                                    start=False, stop=True)
                                nc.scalar.activation(
                                    V[:, sb, ch * 512:(ch + 1) * 512],
                                    ps[sb][:], SILU)

                # ---- attention (ih-outer) with f2 sb0-3 interleaved into
                # the ih=1 pass; f2 sb4-7 after ----
                with (
                    tc.tile_pool(name="attnp", bufs=4) as attnp,
                    tc.tile_pool(name="psav", bufs=2, space="PSUM") as psav,
                    tc.tile_pool(name="pssc", bufs=4, space="PSUM") as pssc,
                    tc.tile_pool(name="psf2", bufs=2, space="PSUM") as psf2,
                    tc.tile_pool(name="w2p", bufs=4) as w2p,
                    tc.tile_pool(name="stgp", bufs=3) as stgp,
                ):
                    def emit_attention(h, ih):
                        njb = (4 * ih + 4) if causal else 8
                        at = at_tiles[h]
                        avp = psav.tile([128, 512], f32, tag="av",
                                        name=f"av{h}_{ih}")
                        chunks = [list(range(j, min(j + 2, njb)))
                                  for j in range(0, njb, 2)]
                        att_tiles = {}

                        def emit_scores(ch_):
                            for jb in ch_:
                                scp = pssc.tile([128, 512], f32, tag="sc",
                                                name=f"sc{h}_{ih}_{jb}")
                                nc.tensor.matmul(
                                    scp[:],
                                    lhsT=KTt[:, h, jb * 128:(jb + 1) * 128],
                                    rhs=QT[:, h, ih * 512:(ih + 1) * 512],
                                    start=True, stop=True)
                                att = attnp.tile([128, 512], mmdt, tag="attn",
                                                 name=f"at{h}_{ih}_{jb}")
                                d0 = ih * 512 - jb * 128 + MAXLEN - 1
                                nc.vector.scalar_tensor_tensor(
                                    att[:], scp[:], SCALE, at[:, d0:d0 + 512],
                                    op0=MULT, op1=ADD)
                                nc.scalar.activation(att[:], att[:], SILU)
                                if not causal:
                                    nc.vector.tensor_mul(
                                        att[:], att[:],
                                        mask_t[:, jb, ih * 512:(ih + 1) * 512])
                                att_tiles[jb] = att

                        emit_scores(chunks[0])
                        for ci, ch_ in enumerate(chunks):
                            if ci + 1 < len(chunks):
                                emit_scores(chunks[ci + 1])
                            for jb in ch_:
                                nc.tensor.matmul(
                                    avp[:],
                                    lhsT=V[:, jb, h * HD:(h + 1) * HD],
                                    rhs=att_tiles.pop(jb)[:],
                                    start=(jb == 0), stop=(jb == njb - 1))
                        nc.vector.tensor_mul(
                            gatedT[:, h, ih * 512:(ih + 1) * 512],
                            avp[:],
                            UT[:, h, ih * 512:(ih + 1) * 512])

                    def emit_f2_block(w2t, n, sb):
                        ps = psf2.tile([128, 512], f32, tag="f2",
                                       name=f"f2_{n}_{sb}")
                        for cb in range(NHL):
                            nc.tensor.matmul(
                                ps[:],
                                lhsT=gatedT[:, cb, sb * 128:(sb + 1) * 128],
                                rhs=w2t[:, cb, :],
                                start=(cb == 0), stop=(cb == NHL - 1))
                        st = stgp.tile([128, 512], f32, tag="st",
                                       name=f"st{n}_{sb}")
                        nc.vector.tensor_copy(st[:], ps[:])
                        nc.sync.dma_start(
                            out[sb * 128:(sb + 1) * 128,
                                n * 512:(n + 1) * 512], st[:])

                    for h in range(NHL):
                        emit_attention(h, 0)

                    w2a = []
                    for n in range(4):
                        t = w2p.tile([128, NHL, 512], mmdt, tag="w2",
                                     name=f"w2a{n}")
                        nc.sync.dma_start(t[:],
                                          wf2r[:, :, n * 512:(n + 1) * 512])
                        w2a.append(t)

                    fa = [(n, sb) for n in range(4) for sb in range(4)]
                    w2b = []
                    for i in range(NHL):
                        emit_attention(i, 1)
                        for n, sb in fa[2 * i:2 * (i + 1)]:
                            emit_f2_block(w2a[n], n, sb)
                        if i % 2 == 1:
                            # column i//2 of part A is done - prefetch its
                            # part-B replacement into the freed slot
                            t = w2p.tile([128, NHL, 512], mmdt, tag="w2",
                                         name=f"w2b{i // 2}")
                            nc.gpsimd.dma_start(
                                t[:], wf2r[:, :, (i // 2) * 512:
                                           (i // 2 + 1) * 512])
                            w2b.append(t)

                    for n in range(4):
                        for sb in range(4, 8):
                            emit_f2_block(w2b[n], n, sb)

    nc.compile()
    return nc


def _host_shards(query, key, value, attn_mask, Wq, bq, Wk, bk, Wv, bv,
                 Wu, bu, Wf2, rel_table, causal, mm_dt=None):
    """Build the per-core input maps."""
    import ml_dtypes
    npdt = (np.dtype(ml_dtypes.bfloat16) if (mm_dt or MM_DT) == "bf16"
            else np.float32)
    _ONES128 = np.ones((1, 128)).astype(npdt)
    in_maps = []
    # precompute per-head-group weight slices once (shared by 4 cores each)
    gdata = []
    for g in range(HGRP):
        c0, c1 = g * NHL * HD, (g + 1) * NHL * HD
        wq_c = np.ascontiguousarray(Wq[:, c0:c1]).astype(npdt)
        wk_c = np.ascontiguousarray(Wk[:, c0:c1]).astype(npdt)
        wv_c = np.ascontiguousarray(Wv[:, c0:c1]).astype(npdt)
        wu_c = np.ascontiguousarray(Wu[:, c0:c1]).astype(npdt)
        wf2_c = np.ascontiguousarray(Wf2[c0:c1, :]).astype(npdt)
        bq_c = np.ascontiguousarray(bq[c0:c1].reshape(NHL, 128).T)
        bk_c = np.ascontiguousarray(bk[c0:c1].reshape(NHL, 128).T)
        bu_c = np.ascontiguousarray(bu[c0:c1].reshape(NHL, 128).T)
        bv_c = np.ascontiguousarray(bv[c0:c1][None, :]).astype(npdt)
        # atab[h, r, y] = table[y - r, g*NHL + h]; for the causal variant the
        # table is pre-divided by SCALE and masked entries (m < MAXLEN-1,
        # i.e. key index > query index) are -1e5 so silu gives exactly 0.
        y = np.arange(2047)[None, :]
        r = np.arange(128)[:, None]
        idx = y - r                      # [128, 2047]
        valid = (idx >= 0) & (idx <= 2 * MAXLEN - 2)
        idxc = np.clip(idx, 0, 2 * MAXLEN - 2)
        cols = rel_table[:, g * NHL:(g + 1) * NHL]   # [2047, NHL]
        import ml_dtypes as _mld
        if causal:
            cols = np.where(np.arange(2047)[:, None] >= MAXLEN - 1, cols,
                            np.float32(-1e5))
            at = np.where(valid[:, :, None], cols[idxc], np.float32(-1e5))
        else:
            at = cols[idxc] * valid[:, :, None]
        atab_c = np.ascontiguousarray(
            at.transpose(2, 0, 1)).astype(_mld.bfloat16)
        gdata.append((wq_c, wk_c, wv_c, wu_c, wf2_c, bq_c, bk_c, bu_c,
                      bv_c, atab_c))

    for c in range(NCORES):
        b, g = c // HGRP, c % HGRP
        (wq_c, wk_c, wv_c, wu_c, wf2_c, bq_c, bk_c, bu_c, bv_c,
         atab_c) = gdata[g]
        m = {
            "qT": np.ascontiguousarray(query[b].T).astype(npdt),
            "kT": np.ascontiguousarray(key[b].T).astype(npdt),
            "vT": np.ascontiguousarray(value[b].T).astype(npdt),
            "wq": wq_c, "wk": wk_c, "wv": wv_c, "wu": wu_c, "wf2": wf2_c,
            "bq": bq_c, "bk": bk_c, "bu": bu_c, "bv": bv_c, "atab": atab_c,
            "ones1": _ONES128,
        }
        mb = attn_mask[b]
        if not causal:
            import ml_dtypes as _mld
            mf = np.empty((128, NHL, S), _mld.bfloat16)
            for jb in range(8):
                mf[:, jb, :] = mb[:, jb * 128:(jb + 1) * 128].T
            m["maskf"] = mf
        in_maps.append(m)
    return in_maps


def kernel(query, key, value, attn_mask, Wq, bq, Wk, bk, Wv, bv, Wu, bu,
           Wf2, bf2, rel_table):
    global LAST_EXEC_NS
    query = np.asarray(query, np.float32)
    key = np.asarray(key, np.float32)
    value = np.asarray(value, np.float32)
    attn_mask = np.asarray(attn_mask, bool)
    Wq, bq = np.asarray(Wq, np.float32), np.asarray(bq, np.float32)
    Wk, bk = np.asarray(Wk, np.float32), np.asarray(bk, np.float32)
    Wv, bv = np.asarray(Wv, np.float32), np.asarray(bv, np.float32)
    Wu, bu = np.asarray(Wu, np.float32), np.asarray(bu, np.float32)
    Wf2, bf2 = np.asarray(Wf2, np.float32), np.asarray(bf2, np.float32)
    rel_table = np.asarray(rel_table, np.float32)

    tril = np.tril(np.ones((S, S), bool))
    causal = all(np.array_equal(attn_mask[b], tril) for b in range(B))

    key_ = (causal, MM_DT)
    if key_ not in _CACHE:
        _CACHE[key_] = _build(causal)
    nc = _CACHE[key_]

    in_maps = _host_shards(query, key, value, attn_mask, Wq, bq, Wk, bk,
                           Wv, bv, Wu, bu, Wf2, rel_table, causal)
    res = run_bass_kernel_spmd(nc, in_maps, list(range(NCORES)), trace=TRACE)
    if res.exec_time_ns is not None:
        LAST_EXEC_NS = res.exec_time_ns

    outp = np.empty((B, S, H), np.float32)
    for b in range(B):
        outp[b] = (res.results[2 * b]["out"] + res.results[2 * b + 1]["out"]
                   + bf2[None, :])
    return outp


# revision 25
# speedup vs baseline: 1.0420x; 1.0060x over previous
"""Trainium2 Bass kernel for nn_BaselineModel_35175782154746 (dense transformer
block with SiLU attention + relative-position bias).

Sharding: 8 NeuronCores = 4 batches x 2 head-groups (8 heads each).
Each core computes, for its (batch b, head-group g):
    U, Q, K, V projections (columns g*1024:(g+1)*1024 of Wu/Wq/Wk/Wv),
    SiLU attention with rel-pos bias for its 8 heads,
    gated = out * U, partial = gated @ Wf2[g*1024:(g+1)*1024, :].
Host reduces: out[b] = partial[2b] + partial[2b+1] + bf2.

All matmuls run with bf16 operands (fp32 PSUM accumulation) at N=512 moving
dim — the TensorEngine's full-rate path. Layouts keep the contraction dim on
SBUF partitions (inputs pre-transposed on host). The rel-pos bias is added in
PSUM via an identity-matmul of a host-built shifted table (pre-divided by the
attention scale so ACT's native scale finishes scores = silu(scale*(QK+bias));
for the causal variant the mask is folded into that table as -1e5, which silu
maps to an exact 0.0 in fp32). A dense-mask fallback variant handles any
non-causal attn_mask exactly.
"""

import sys
import os

for _p in ("/root/.axon_site/_ro/trn_rl_repo", "/opt/trn_rl_repo"):
    if os.path.isdir(_p) and _p not in sys.path:
        sys.path.append(_p)

import numpy as np

import concourse.bass as bass
import concourse.mybir as mybir
import concourse.tile as tile
from concourse import bacc
from concourse.bass_utils import run_bass_kernel_spmd

B, S, H, NH, MAXLEN = 4, 1024, 2048, 16, 1024
HD = H // NH            # 128
NHL = 8                 # heads per core (local)
HGRP = 2                # head groups
NCORES = 8
KT16 = H // 128         # 16 k-tiles for the H contraction
SCALE = float(HD) ** -0.5

f32 = mybir.dt.float32
f32r = mybir.dt.float32r
bf16 = mybir.dt.bfloat16
SILU = mybir.ActivationFunctionType.Silu
MULT = mybir.AluOpType.mult
ADD = mybir.AluOpType.add

TRACE = False
LAST_EXEC_NS = None
MM_DT = "bf16"          # "bf16" or "f32r" matmul operand dtype
_CACHE = {}


def _build(causal: bool, mm_dt=None):
    mmdt = {"bf16": bf16, "f32r": f32r}[mm_dt or MM_DT]
    nc = bacc.Bacc("TRN2", target_bir_lowering=False, debug=False,
                   num_devices=NCORES)

    def din(name, shape, dt=f32):
        return nc.dram_tensor(name, shape, dt, kind="ExternalInput").ap()

    qT = din("qT", [H, S], mmdt)
    kT = din("kT", [H, S], mmdt)
    vT = din("vT", [H, S], mmdt)
    wq = din("wq", [H, NHL * HD], mmdt)
    wk = din("wk", [H, NHL * HD], mmdt)
    wv = din("wv", [H, NHL * HD], mmdt)
    wu = din("wu", [H, NHL * HD], mmdt)
    wf2 = din("wf2", [NHL * HD, H], mmdt)
    bq = din("bq", [128, NHL])
    bk = din("bk", [128, NHL])
    bu = din("bu", [128, NHL])
    bv = din("bv", [1, NHL * HD], mmdt)
    ones1 = din("ones1", [1, 128], mmdt)
    atab = din("atab", [NHL, 128, 2047], bf16)
    if not causal:
        maskf = din("maskf", [128, NHL, S], bf16)
    out = nc.dram_tensor("out", [S, H], f32, kind="ExternalOutput").ap()

    with tile.TileContext(nc) as tc:
        with (
            tc.tile_pool(name="const", bufs=1) as constp,
            tc.tile_pool(name="gatedp", bufs=1) as gatedp,
        ):
            bq_t = constp.tile([128, NHL], f32, tag="bq")
            bk_t = constp.tile([128, NHL], f32, tag="bk")
            bu_t = constp.tile([128, NHL], f32, tag="bu")
            bv_t = constp.tile([1, NHL * HD], mmdt, tag="bv")
            ones_t = constp.tile([1, 128], mmdt, tag="ones1")

            gatedT = gatedp.tile([128, NHL, S], mmdt, tag="gatedT")
            wf2r = wf2.rearrange("(cb p) n -> p cb n", p=128)

            with tc.tile_pool(name="attres", bufs=1) as attres:
                UT = attres.tile([128, NHL, S], bf16, tag="UT")
                QT = attres.tile([128, NHL, S], mmdt, tag="QT")
                KTt = attres.tile([128, NHL, S], mmdt, tag="KT")
                V = attres.tile([128, NHL, S], mmdt, tag="V")
                at_tiles = [attres.tile([128, 2047], bf16,
                                        tag=f"atab{h}", name=f"atab{h}")
                            for h in range(NHL)]
                if not causal:
                    mask_t = attres.tile([128, NHL, S], bf16, tag="mask")

                with tc.tile_pool(name="inres", bufs=1) as inres:
                    qres = inres.tile([128, KT16, S], mmdt, tag="qres")
                    kres = inres.tile([128, KT16, S], mmdt, tag="kres")
                    # vres shares qres's slot: qres's last read is the Q
                    # phase, V runs last, so the vres load lands during K.
                    vres = inres.tile([128, KT16, S], mmdt, tag="qres",
                                      name="vres")
                    for k in range(KT16):
                        nc.sync.dma_start(qres[:, k, :],
                                          qT[k * 128:(k + 1) * 128, :])
                    nc.sync.dma_start(bu_t[:], bu[:])
                    nc.sync.dma_start(bq_t[:], bq[:])
                    nc.sync.dma_start(bk_t[:], bk[:])
                    nc.sync.dma_start(bv_t[:], bv[:])
                    nc.sync.dma_start(ones_t[:], ones1[:])
                    for k in range(KT16):
                        nc.sync.dma_start(kres[:, k, :],
                                          kT[k * 128:(k + 1) * 128, :])
                    for k in range(KT16):
                        nc.sync.dma_start(vres[:, k, :],
                                          vT[k * 128:(k + 1) * 128, :])
                    for h in range(NHL):
                        nc.sync.dma_start(at_tiles[h][:], atab[h])
                    if not causal:
                        nc.sync.dma_start(mask_t[:], maskf[:])

                    with (
                        tc.tile_pool(name="win", bufs=6 if causal else 4) as winp,
                        tc.tile_pool(name="pps", bufs=1, space="PSUM") as ppsum,
                    ):
                        # ---- projections U, Q, K ([HD, S] transposed) ----
                        for wdram, xres, btile, outtile in (
                            (wu, qres, bu_t, UT),
                            (wq, qres, bq_t, QT),
                            (wk, kres, bk_t, KTt),
                        ):
                            for ih in range(2):
                                ps = [ppsum.tile([128, 512], f32, tag=f"pp{h}",
                                                 name=f"pp{h}")
                                      for h in range(NHL)]
                                for k in range(KT16):
                                    wt = winp.tile([128, NHL * HD], mmdt,
                                                   tag="win")
                                    nc.gpsimd.dma_start(
                                        wt[:], wdram[k * 128:(k + 1) * 128, :])
                                    for h in range(NHL):
                                        nc.tensor.matmul(
                                            ps[h][:],
                                            lhsT=wt[:, h * HD:(h + 1) * HD],
                                            rhs=xres[:, k,
                                                     ih * 512:(ih + 1) * 512],
                                            start=(k == 0),
                                            stop=(k == KT16 - 1))
                                for h in range(NHL):
                                    nc.scalar.activation(
                                        outtile[:, h, ih * 512:(ih + 1) * 512],
                                        ps[h][:], SILU, bias=btile[:, h:h + 1])

                        # ---- projection V (natural layout [S, NHL*HD]) ----
                        for ch in range(2):
                            ps = [ppsum.tile([128, 512], f32, tag=f"pp{sb}",
                                             name=f"ppv{sb}")
                                  for sb in range(8)]
                            for k in range(KT16):
                                wt = winp.tile([128, 512], mmdt, tag="wvin")
                                nc.gpsimd.dma_start(
                                    wt[:], wv[k * 128:(k + 1) * 128,
                                              ch * 512:(ch + 1) * 512])
                                for sb in range(8):
                                    nc.tensor.matmul(
                                        ps[sb][:],
                                        lhsT=vres[:, k, sb * 128:(sb + 1) * 128],
                                        rhs=wt[:],
                                        start=(k == 0), stop=False)
                            for sb in range(8):
                                nc.tensor.matmul(
                                    ps[sb][:],
                                    lhsT=ones_t[:],
                                    rhs=bv_t[:, ch * 512:(ch + 1) * 512],
                                    start=False, stop=True)
                                nc.scalar.activation(
                                    V[:, sb, ch * 512:(ch + 1) * 512],
                                    ps[sb][:], SILU)

                # ---- attention (ih-outer) with f2 sb0-3 interleaved into
                # the ih=1 pass; f2 sb4-7 after ----
                with (
                    tc.tile_pool(name="attnp", bufs=4) as attnp,
                    tc.tile_pool(name="psav", bufs=2, space="PSUM") as psav,
                    tc.tile_pool(name="pssc", bufs=4, space="PSUM") as pssc,
                    tc.tile_pool(name="psf2", bufs=2, space="PSUM") as psf2,
                    tc.tile_pool(name="w2p", bufs=8) as w2p,
                    tc.tile_pool(name="stgp", bufs=3) as stgp,
                ):
                    def emit_attention(h, ih):
                        njb = (4 * ih + 4) if causal else 8
                        at = at_tiles[h]
                        avp = psav.tile([128, 512], f32, tag="av",
                                        name=f"av{h}_{ih}")
                        chunks = [list(range(j, min(j + 2, njb)))
                                  for j in range(0, njb, 2)]
                        att_tiles = {}

                        def emit_scores(ch_):
                            for jb in ch_:
                                scp = pssc.tile([128, 512], f32, tag="sc",
                                                name=f"sc{h}_{ih}_{jb}")
                                nc.tensor.matmul(
                                    scp[:],
                                    lhsT=KTt[:, h, jb * 128:(jb + 1) * 128],
                                    rhs=QT[:, h, ih * 512:(ih + 1) * 512],
                                    start=True, stop=True)
                                att = attnp.tile([128, 512], mmdt, tag="attn",
                                                 name=f"at{h}_{ih}_{jb}")
                                d0 = ih * 512 - jb * 128 + MAXLEN - 1
                                nc.vector.scalar_tensor_tensor(
                                    att[:], scp[:], SCALE, at[:, d0:d0 + 512],
                                    op0=MULT, op1=ADD)
                                nc.scalar.activation(att[:], att[:], SILU)
                                if not causal:
                                    nc.vector.tensor_mul(
                                        att[:], att[:],
                                        mask_t[:, jb, ih * 512:(ih + 1) * 512])
                                att_tiles[jb] = att

                        emit_scores(chunks[0])
                        for ci, ch_ in enumerate(chunks):
                            if ci + 1 < len(chunks):
                                emit_scores(chunks[ci + 1])
                            for jb in ch_:
                                nc.tensor.matmul(
                                    avp[:],
                                    lhsT=V[:, jb, h * HD:(h + 1) * HD],
                                    rhs=att_tiles.pop(jb)[:],
                                    start=(jb == 0), stop=(jb == njb - 1))
                        nc.vector.tensor_mul(
                            gatedT[:, h, ih * 512:(ih + 1) * 512],
                            avp[:],
                            UT[:, h, ih * 512:(ih + 1) * 512])

                    def emit_f2_block(w2t, n, sb):
                        ps = psf2.tile([128, 512], f32, tag="f2",
                                       name=f"f2_{n}_{sb}")
                        for cb in range(NHL):
                            nc.tensor.matmul(
                                ps[:],
                                lhsT=gatedT[:, cb, sb * 128:(sb + 1) * 128],
                                rhs=w2t[:, cb, :],
                                start=(cb == 0), stop=(cb == NHL - 1))
                        st = stgp.tile([128, 512], f32, tag="st",
                                       name=f"st{n}_{sb}")
                        nc.vector.tensor_copy(st[:], ps[:])
                        nc.sync.dma_start(
                            out[sb * 128:(sb + 1) * 128,
                                n * 512:(n + 1) * 512], st[:])

                    for h in range(NHL):
                        emit_attention(h, 0)

                    w2a = []
                    for n in range(4):
                        t = w2p.tile([128, NHL, 512], mmdt, tag="w2",
                                     name=f"w2a{n}")
                        nc.sync.dma_start(t[:],
                                          wf2r[:, :, n * 512:(n + 1) * 512])
                        w2a.append(t)

                    fa = [(n, sb) for n in range(4) for sb in range(4)]
                    w2b = []
                    for i in range(NHL):
                        emit_attention(i, 1)
                        for n, sb in fa[2 * i:2 * (i + 1)]:
                            emit_f2_block(w2a[n], n, sb)
                        if i % 2 == 1:
                            # column i//2 of part A is done - prefetch its
                            # part-B replacement into the freed slot
                            t = w2p.tile([128, NHL, 512], mmdt, tag="w2",
                                         name=f"w2b{i // 2}")
                            nc.gpsimd.dma_start(
                                t[:], wf2r[:, :, (i // 2) * 512:
                                           (i // 2 + 1) * 512])
                            w2b.append(t)

                    for n in range(4):
                        for sb in range(4, 8):
                            emit_f2_block(w2b[n], n, sb)

    nc.compile()
    return nc


def _host_shards(query, key, value, attn_mask, Wq, bq, Wk, bk, Wv, bv,
                 Wu, bu, Wf2, rel_table, causal, mm_dt=None):
    """Build the per-core input maps."""
    import ml_dtypes
    npdt = (np.dtype(ml_dtypes.bfloat16) if (mm_dt or MM_DT) == "bf16"
            else np.float32)
    _ONES128 = np.ones((1, 128)).astype(npdt)
    in_maps = []
    # precompute per-head-group weight slices once (shared by 4 cores each)
    gdata = []
    for g in range(HGRP):
        c0, c1 = g * NHL * HD, (g + 1) * NHL * HD
        wq_c = np.ascontiguousarray(Wq[:, c0:c1]).astype(npdt)
        wk_c = np.ascontiguousarray(Wk[:, c0:c1]).astype(npdt)
        wv_c = np.ascontiguousarray(Wv[:, c0:c1]).astype(npdt)
        wu_c = np.ascontiguousarray(Wu[:, c0:c1]).astype(npdt)
        wf2_c = np.ascontiguousarray(Wf2[c0:c1, :]).astype(npdt)
        bq_c = np.ascontiguousarray(bq[c0:c1].reshape(NHL, 128).T)
        bk_c = np.ascontiguousarray(bk[c0:c1].reshape(NHL, 128).T)
        bu_c = np.ascontiguousarray(bu[c0:c1].reshape(NHL, 128).T)
        bv_c = np.ascontiguousarray(bv[c0:c1][None, :]).astype(npdt)
        # atab[h, r, y] = table[y - r, g*NHL + h]; for the causal variant the
        # table is pre-divided by SCALE and masked entries (m < MAXLEN-1,
        # i.e. key index > query index) are -1e5 so silu gives exactly 0.
        y = np.arange(2047)[None, :]
        r = np.arange(128)[:, None]
        idx = y - r                      # [128, 2047]
        valid = (idx >= 0) & (idx <= 2 * MAXLEN - 2)
        idxc = np.clip(idx, 0, 2 * MAXLEN - 2)
        cols = rel_table[:, g * NHL:(g + 1) * NHL]   # [2047, NHL]
        import ml_dtypes as _mld
        if causal:
            cols = np.where(np.arange(2047)[:, None] >= MAXLEN - 1, cols,
                            np.float32(-1e5))
            at = np.where(valid[:, :, None], cols[idxc], np.float32(-1e5))
        else:
            at = cols[idxc] * valid[:, :, None]
        atab_c = np.ascontiguousarray(
            at.transpose(2, 0, 1)).astype(_mld.bfloat16)
        gdata.append((wq_c, wk_c, wv_c, wu_c, wf2_c, bq_c, bk_c, bu_c,
                      bv_c, atab_c))

    for c in range(NCORES):
        b, g = c // HGRP, c % HGRP
        (wq_c, wk_c, wv_c, wu_c, wf2_c, bq_c, bk_c, bu_c, bv_c,
         atab_c) = gdata[g]
        m = {
            "qT": np.ascontiguousarray(query[b].T).astype(npdt),
            "kT": np.ascontiguousarray(key[b].T).astype(npdt),
            "vT": np.ascontiguousarray(value[b].T).astype(npdt),
            "wq": wq_c, "wk": wk_c, "wv": wv_c, "wu": wu_c, "wf2": wf2_c,
            "bq": bq_c, "bk": bk_c, "bu": bu_c, "bv": bv_c, "atab": atab_c,
            "ones1": _ONES128,
        }
        mb = attn_mask[b]
        if not causal:
            import ml_dtypes as _mld
            mf = np.empty((128, NHL, S), _mld.bfloat16)
            for jb in range(8):
                mf[:, jb, :] = mb[:, jb * 128:(jb + 1) * 128].T
            m["maskf"] = mf
        in_maps.append(m)
    return in_maps


def kernel(query, key, value, attn_mask, Wq, bq, Wk, bk, Wv, bv, Wu, bu,
           Wf2, bf2, rel_table):
    global LAST_EXEC_NS
    query = np.asarray(query, np.float32)
    key = np.asarray(key, np.float32)
    value = np.asarray(value, np.float32)
    attn_mask = np.asarray(attn_mask, bool)
    Wq, bq = np.asarray(Wq, np.float32), np.asarray(bq, np.float32)
    Wk, bk = np.asarray(Wk, np.float32), np.asarray(bk, np.float32)
    Wv, bv = np.asarray(Wv, np.float32), np.asarray(bv, np.float32)
    Wu, bu = np.asarray(Wu, np.float32), np.asarray(bu, np.float32)
    Wf2, bf2 = np.asarray(Wf2, np.float32), np.asarray(bf2, np.float32)
    rel_table = np.asarray(rel_table, np.float32)

    tril = np.tril(np.ones((S, S), bool))
    causal = all(np.array_equal(attn_mask[b], tril) for b in range(B))

    key_ = (causal, MM_DT)
    if key_ not in _CACHE:
        _CACHE[key_] = _build(causal)
    nc = _CACHE[key_]

    in_maps = _host_shards(query, key, value, attn_mask, Wq, bq, Wk, bk,
                           Wv, bv, Wu, bu, Wf2, rel_table, causal)
    res = run_bass_kernel_spmd(nc, in_maps, list(range(NCORES)), trace=TRACE)
    if res.exec_time_ns is not None:
        LAST_EXEC_NS = res.exec_time_ns

    outp = np.empty((B, S, H), np.float32)
    for b in range(B):
        outp[b] = (res.results[2 * b]["out"] + res.results[2 * b + 1]["out"]
                   + bf2[None, :])
    return outp
